# revision 2
# baseline (speedup 1.0000x reference)
"""Trainium2 Bass kernel for nn_Attention_58437325029959 (sparse_attention).

Reference computation (per batch b, with m = d = 128, n = 2048):
    Vs = V / m
    Q1 = 2 Vs Vs^T;  P = -2 Vs Q^T + lam/m        (P viewed as [n, m])
    50 ADMM iterations of the box QP  min 0.5 x^T Q1 x + P x, 0 <= x <= 1
    xb = (z_50 > 0.5);  out = (xb / rowsum(xb)) @ Vs

Algebraic form used on device (exactly equivalent in exact arithmetic):
    M_inv = inv(Q1 + I);  A = 2 M_inv - I;  B = I - M_inv
    C^T   = (-2 M_inv Vs) Q^T + (lam/m) (M_inv 1) 1^T        [m, n]
    t_1   = -C^T;   t_{k+1} = A z_k + B t_k - C^T,  z_k = clip(t_k)
    out^T = (Vs^T xb^T) / colsum(xb^T),  xb^T = (t_50 > 0.5)

Sharding: one batch element per NeuronCore (8 cores).  All state is kept
transposed: [m=128 partitions, n=2048 free] per core.
"""

import numpy as np

import concourse.bass as bass
import concourse.mybir as mybir
import concourse.tile as tile
from concourse import bacc
from concourse.bass_utils import run_bass_kernel_spmd

LAMBDA = 0.1
RHO = 1.0
N_ITERS = 50

B, N, D = 8, 2048, 128
M = 128
N_CORES = 8
CHUNK = 512
NCHUNKS = N // CHUNK

F32 = mybir.dt.float32
F32R = mybir.dt.float32r

# 'f32' (exact, 4 cyc/row) or 'f32r' (1 cyc/row at N>=256, reduced precision)
import os as _os
MM_DTYPE = _os.environ.get("KERNEL_MM_DTYPE", "f32")

_compiled = {}


def _mm_ap(ap):
    if MM_DTYPE == "f32r":
        return ap.bitcast(F32R)
    return ap


def _build():
    """Build (and cache) the Bass program. Same program on all 8 cores."""
    key = (MM_DTYPE,)
    if key in _compiled:
        return _compiled[key]

    nc = bacc.Bacc("TRN2", target_bir_lowering=False, debug=False,
                   num_devices=N_CORES)

    ct_d = nc.dram_tensor("ct", [M, N], F32, kind="ExternalInput").ap()
    at_d = nc.dram_tensor("at", [M, M], F32, kind="ExternalInput").ap()
    bt_d = nc.dram_tensor("bt", [M, M], F32, kind="ExternalInput").ap()
    vs_d = nc.dram_tensor("vs", [M, D], F32, kind="ExternalInput").ap()
    out_d = nc.dram_tensor("outT", [D, N], F32, kind="ExternalOutput").ap()

    with tile.TileContext(nc) as tc:
        with (
            tc.tile_pool(name="sb", bufs=1) as sb,
            tc.tile_pool(name="ps", bufs=2, space="PSUM") as psp,
        ):
            CT = sb.tile([M, N], F32)
            AT = sb.tile([M, M], F32)
            BT = sb.tile([M, M], F32)
            VS = sb.tile([M, D], F32)
            ONES = sb.tile([M, M], F32)
            nc.sync.dma_start(CT[:], ct_d)
            nc.sync.dma_start(AT[:], at_d)
            nc.sync.dma_start(BT[:], bt_d)
            nc.sync.dma_start(VS[:], vs_d)
            nc.gpsimd.memset(ONES[:], 1.0)

            T = sb.tile([M, N], F32)
            Z = sb.tile([M, N], F32)

            # t_1 = -C^T ;  z_1 = clip(t_1)
            nc.vector.tensor_scalar(T[:], CT[:], -1.0, None,
                                    mybir.AluOpType.mult)
            nc.gpsimd.tensor_scalar(Z[:], T[:], 0.0, 1.0,
                                    mybir.AluOpType.max, mybir.AluOpType.min)

            for _ in range(N_ITERS - 1):
                ps = psp.tile([M, N], F32, tag="ps")
                for c in range(NCHUNKS):
                    sl = bass.ts(c, CHUNK)
                    nc.tensor.matmul(ps[:, sl], _mm_ap(AT[:]), _mm_ap(Z[:, sl]),
                                     start=True, stop=False)
                for c in range(NCHUNKS):
                    sl = bass.ts(c, CHUNK)
                    nc.tensor.matmul(ps[:, sl], _mm_ap(BT[:]), _mm_ap(T[:, sl]),
                                     start=False, stop=True)
                for c in range(NCHUNKS):
                    sl = bass.ts(c, CHUNK)
                    nc.vector.tensor_tensor(T[:, sl], ps[:, sl], CT[:, sl],
                                            mybir.AluOpType.subtract)
                    nc.gpsimd.tensor_scalar(Z[:, sl], T[:, sl], 0.0, 1.0,
                                            mybir.AluOpType.max,
                                            mybir.AluOpType.min)

            # xb^T = (t_50 > 0.5)  as 1.0 / 0.0
            XB = sb.tile([M, N], F32)
            nc.vector.tensor_scalar(XB[:], T[:], 0.5, None,
                                    mybir.AluOpType.is_gt)

            # numerator: Vs^T xb^T  -> [d, n]; denominator: colsums broadcast
            pv = psp.tile([M, N], F32, tag="ps")
            pc = psp.tile([M, N], F32, tag="ps")
            for c in range(NCHUNKS):
                sl = bass.ts(c, CHUNK)
                nc.tensor.matmul(pv[:, sl], _mm_ap(VS[:]), _mm_ap(XB[:, sl]),
                                 start=True, stop=True)
            for c in range(NCHUNKS):
                sl = bass.ts(c, CHUNK)
                nc.tensor.matmul(pc[:, sl], _mm_ap(ONES[:]), _mm_ap(XB[:, sl]),
                                 start=True, stop=True)

            DEN = sb.tile([M, N], F32)
            nc.vector.tensor_scalar(DEN[:], pc[:], 1e-10, None,
                                    mybir.AluOpType.add)
            # 1/den = exp(-ln(den)) on the scalar engine
            LNV = sb.tile([M, N], F32)
            nc.scalar.activation(LNV[:], DEN[:], mybir.ActivationFunctionType.Ln)
            REC = sb.tile([M, N], F32)
            nc.scalar.activation(REC[:], LNV[:],
                                 mybir.ActivationFunctionType.Exp, scale=-1.0)

            OUT = sb.tile([D, N], F32)
            nc.vector.tensor_tensor(OUT[:], pv[:], REC[:],
                                    mybir.AluOpType.mult)
            nc.sync.dma_start(out_d, OUT[:])

    nc.compile()
    _compiled[key] = nc
    return nc


def _host_precompute(Q, V):
    """Per-batch constants in float64, cast to float32."""
    b = Q.shape[0]
    m = V.shape[1]
    in_maps = []
    for bi in range(b):
        Vs64 = V[bi].astype(np.float64) / m
        eye = np.eye(m)
        Q1 = 2.0 * (Vs64 @ Vs64.T)
        Minv = np.linalg.inv(Q1 + RHO * eye)
        A = 2.0 * Minv - eye
        Bm = eye - Minv
        W = -2.0 * (Minv @ Vs64)
        c0 = (LAMBDA / m) * Minv.sum(axis=1)
        CT = W @ Q[bi].astype(np.float64).T + c0[:, None]
        # matmul computes lhsT.T @ rhs -> pass explicit transposes
        in_maps.append({
            "ct": np.ascontiguousarray(CT, dtype=np.float32),
            "at": np.ascontiguousarray(A.T, dtype=np.float32),
            "bt": np.ascontiguousarray(Bm.T, dtype=np.float32),
            # final product lhsT = Vs (out^T = Vs^T @ xb^T); match the
            # reference's f32 V/m rounding exactly
            "vs": np.ascontiguousarray(V[bi].astype(np.float32) / np.float32(m)),
        })
    return in_maps


def kernel(Q, V):
    Q = np.asarray(Q, dtype=np.float32)
    V = np.asarray(V, dtype=np.float32)
    nc = _build()
    in_maps = _host_precompute(Q, V)
    res = run_bass_kernel_spmd(nc, in_maps, list(range(N_CORES)))
    out = np.empty((B, N, D), dtype=np.float32)
    for bi in range(B):
        out[bi] = res.results[bi]["outT"].T
    return out


# revision 3
# speedup vs baseline: 4.1387x; 4.1387x over previous
"""Trainium2 Bass kernel for nn_Attention_58437325029959 (sparse_attention).

Reference computation (per batch b, with m = d = 128, n = 2048):
    Vs = V / m
    Q1 = 2 Vs Vs^T;  P = -2 Vs Q^T + lam/m        (P viewed as [n, m])
    50 ADMM iterations of the box QP  min 0.5 x^T Q1 x + P x, 0 <= x <= 1
    xb = (z_50 > 0.5);  out = (xb / rowsum(xb)) @ Vs

Algebraic form used on device (exactly equivalent in exact arithmetic):
    M_inv = inv(Q1 + I);  A = 2 M_inv - I;  B = I - M_inv
    C^T   = (-2 M_inv Vs) Q^T + (lam/m) (M_inv 1) 1^T        [m, n]
    t_1   = -C^T;   t_{k+1} = A z_k + B t_k - C^T,  z_k = clip(t_k)
    out^T = (Vs^T xb^T) / colsum(xb^T),  xb^T = (t_50 > 0.5)

Sharding: one batch element per NeuronCore (8 cores).  All state is kept
transposed: [m=128 partitions, n=2048 free] per core.
"""

import numpy as np

import concourse.bass as bass
import concourse.mybir as mybir
import concourse.tile as tile
from concourse import bacc
from concourse.bass_utils import run_bass_kernel_spmd

LAMBDA = 0.1
RHO = 1.0
N_ITERS = 50

B, N, D = 8, 2048, 128
M = 128
N_CORES = 8
CHUNK = 512
NCHUNKS = N // CHUNK

F32 = mybir.dt.float32
F32R = mybir.dt.float32r

# 'f32' (exact, 4 cyc/row) or 'f32r' (1 cyc/row at N>=256, reduced precision)
import os as _os
MM_DTYPE = _os.environ.get("KERNEL_MM_DTYPE", "f32")

_compiled = {}


def _mm_ap(ap):
    if MM_DTYPE == "f32r":
        return ap.bitcast(F32R)
    return ap


def _build():
    """Build (and cache) the Bass program. Same program on all 8 cores."""
    key = (MM_DTYPE,)
    if key in _compiled:
        return _compiled[key]

    nc = bacc.Bacc("TRN2", target_bir_lowering=False, debug=False,
                   num_devices=N_CORES)

    ct_d = nc.dram_tensor("ct", [M, N], F32, kind="ExternalInput").ap()
    at_d = nc.dram_tensor("at", [M, M], F32, kind="ExternalInput").ap()
    bt_d = nc.dram_tensor("bt", [M, M], F32, kind="ExternalInput").ap()
    vs_d = nc.dram_tensor("vs", [M, D], F32, kind="ExternalInput").ap()
    out_d = nc.dram_tensor("outT", [D, N], F32, kind="ExternalOutput").ap()

    with tile.TileContext(nc) as tc:
        with (
            tc.tile_pool(name="sb", bufs=1) as sb,
            tc.tile_pool(name="ps", bufs=2, space="PSUM") as psp,
        ):
            CT = sb.tile([M, N], F32)
            AT = sb.tile([M, M], F32)
            BT = sb.tile([M, M], F32)
            VS = sb.tile([M, D], F32)
            ONES = sb.tile([M, M], F32)
            nc.sync.dma_start(CT[:], ct_d)
            nc.sync.dma_start(AT[:], at_d)
            nc.sync.dma_start(BT[:], bt_d)
            nc.sync.dma_start(VS[:], vs_d)
            nc.gpsimd.memset(ONES[:], 1.0)

            T = sb.tile([M, N], F32)
            Z = sb.tile([M, N], F32)

            # t_1 = -C^T ;  z_1 = clip(t_1)
            nc.vector.tensor_scalar(T[:], CT[:], -1.0, None,
                                    mybir.AluOpType.mult)
            nc.vector.tensor_scalar(Z[:], T[:], 0.0, 1.0,
                                    mybir.AluOpType.max, mybir.AluOpType.min)

            for _ in range(N_ITERS - 1):
                ps = psp.tile([M, N], F32, tag="ps")
                for c in range(NCHUNKS):
                    sl = bass.ts(c, CHUNK)
                    nc.tensor.matmul(ps[:, sl], _mm_ap(AT[:]), _mm_ap(Z[:, sl]),
                                     start=True, stop=False)
                    nc.tensor.matmul(ps[:, sl], _mm_ap(BT[:]), _mm_ap(T[:, sl]),
                                     start=False, stop=True)
                for c in range(NCHUNKS):
                    sl = bass.ts(c, CHUNK)
                    nc.vector.tensor_tensor(T[:, sl], ps[:, sl], CT[:, sl],
                                            mybir.AluOpType.subtract)
                    nc.vector.tensor_scalar(Z[:, sl], T[:, sl], 0.0, 1.0,
                                            mybir.AluOpType.max,
                                            mybir.AluOpType.min)

            # xb^T = (t_50 > 0.5)  as 1.0 / 0.0
            XB = sb.tile([M, N], F32)
            nc.vector.tensor_scalar(XB[:], T[:], 0.5, None,
                                    mybir.AluOpType.is_gt)

            # numerator: Vs^T xb^T  -> [d, n]; denominator: colsums broadcast
            pv = psp.tile([M, N], F32, tag="ps")
            pc = psp.tile([M, N], F32, tag="ps")
            for c in range(NCHUNKS):
                sl = bass.ts(c, CHUNK)
                nc.tensor.matmul(pv[:, sl], _mm_ap(VS[:]), _mm_ap(XB[:, sl]),
                                 start=True, stop=True)
            for c in range(NCHUNKS):
                sl = bass.ts(c, CHUNK)
                nc.tensor.matmul(pc[:, sl], _mm_ap(ONES[:]), _mm_ap(XB[:, sl]),
                                 start=True, stop=True)

            DEN = sb.tile([M, N], F32)
            nc.vector.tensor_scalar(DEN[:], pc[:], 1e-10, None,
                                    mybir.AluOpType.add)
            # 1/den = exp(-ln(den)) on the scalar engine
            LNV = sb.tile([M, N], F32)
            nc.scalar.activation(LNV[:], DEN[:], mybir.ActivationFunctionType.Ln)
            REC = sb.tile([M, N], F32)
            nc.scalar.activation(REC[:], LNV[:],
                                 mybir.ActivationFunctionType.Exp, scale=-1.0)

            OUT = sb.tile([D, N], F32)
            nc.vector.tensor_tensor(OUT[:], pv[:], REC[:],
                                    mybir.AluOpType.mult)
            nc.sync.dma_start(out_d, OUT[:])

    nc.compile()
    _compiled[key] = nc
    return nc


def _host_precompute(Q, V):
    """Per-batch constants in float64, cast to float32."""
    b = Q.shape[0]
    m = V.shape[1]
    in_maps = []
    for bi in range(b):
        Vs64 = V[bi].astype(np.float64) / m
        eye = np.eye(m)
        Q1 = 2.0 * (Vs64 @ Vs64.T)
        Minv = np.linalg.inv(Q1 + RHO * eye)
        A = 2.0 * Minv - eye
        Bm = eye - Minv
        W = -2.0 * (Minv @ Vs64)
        c0 = (LAMBDA / m) * Minv.sum(axis=1)
        CT = W @ Q[bi].astype(np.float64).T + c0[:, None]
        # matmul computes lhsT.T @ rhs -> pass explicit transposes
        in_maps.append({
            "ct": np.ascontiguousarray(CT, dtype=np.float32),
            "at": np.ascontiguousarray(A.T, dtype=np.float32),
            "bt": np.ascontiguousarray(Bm.T, dtype=np.float32),
            # final product lhsT = Vs (out^T = Vs^T @ xb^T); match the
            # reference's f32 V/m rounding exactly
            "vs": np.ascontiguousarray(V[bi].astype(np.float32) / np.float32(m)),
        })
    return in_maps


def kernel(Q, V):
    Q = np.asarray(Q, dtype=np.float32)
    V = np.asarray(V, dtype=np.float32)
    nc = _build()
    in_maps = _host_precompute(Q, V)
    res = run_bass_kernel_spmd(nc, in_maps, list(range(N_CORES)))
    out = np.empty((B, N, D), dtype=np.float32)
    for bi in range(B):
        out[bi] = res.results[bi]["outT"].T
    return out


# revision 7
# speedup vs baseline: 4.9551x; 1.1972x over previous
"""Trainium2 Bass kernel for nn_Attention_58437325029959 (sparse_attention).

Reference computation (per batch b, with m = d = 128, n = 2048):
    Vs = V / m
    Q1 = 2 Vs Vs^T;  P = -2 Vs Q^T + lam/m        (P viewed as [n, m])
    50 ADMM iterations of the box QP  min 0.5 x^T Q1 x + P x, 0 <= x <= 1
    xb = (z_50 > 0.5);  out = (xb / rowsum(xb)) @ Vs

Algebraic form used on device (exactly equivalent in exact arithmetic):
    M_inv = inv(Q1 + I);  A = 2 M_inv - I;  B = I - M_inv
    C^T   = (-2 M_inv Vs) Q^T + (lam/m) (M_inv 1) 1^T        [m, n]
    t_1   = -C^T;   t_{k+1} = A z_k + B t_k - C^T,  z_k = clip(t_k)
    out^T = (Vs^T xb^T) / colsum(xb^T),  xb^T = (t_50 > 0.5)

Sharding: one batch element per NeuronCore (8 cores).  All state is kept
transposed: [m=128 partitions, n=2048 free] per core.
"""

import numpy as np

import concourse.bass as bass
import concourse.mybir as mybir
import concourse.tile as tile
from concourse import bacc
from concourse.bass_utils import run_bass_kernel_spmd

LAMBDA = 0.1
RHO = 1.0
N_ITERS = 50

B, N, D = 8, 2048, 128
M = 128
N_CORES = 8
CHUNK = 512
NCHUNKS = N // CHUNK

F32 = mybir.dt.float32
F32R = mybir.dt.float32r

# 'f32' (exact, 4 cyc/row) or 'f32r' (1 cyc/row at N>=256, reduced precision)
import os as _os
MM_DTYPE = _os.environ.get("KERNEL_MM_DTYPE", "f32")

_compiled = {}


def _mm_ap(ap):
    if MM_DTYPE == "f32r":
        return ap.bitcast(F32R)
    return ap


def _build():
    """Build (and cache) the Bass program. Same program on all 8 cores."""
    key = (MM_DTYPE,)
    if key in _compiled:
        return _compiled[key]

    nc = bacc.Bacc("TRN2", target_bir_lowering=False, debug=False,
                   num_devices=N_CORES)

    ct_d = nc.dram_tensor("ct", [M, N], F32, kind="ExternalInput").ap()
    at_d = nc.dram_tensor("at", [M, M], F32, kind="ExternalInput").ap()
    bt_d = nc.dram_tensor("bt", [M, M], F32, kind="ExternalInput").ap()
    vs_d = nc.dram_tensor("vs", [M, D], F32, kind="ExternalInput").ap()
    out_d = nc.dram_tensor("outT", [D, N], F32, kind="ExternalOutput").ap()

    with tile.TileContext(nc) as tc:
        with (
            tc.tile_pool(name="sb", bufs=1) as sb,
            tc.tile_pool(name="ps", bufs=2, space="PSUM") as psp,
        ):
            CT = sb.tile([M, N], F32)
            AT = sb.tile([M, M], F32)
            BT = sb.tile([M, M], F32)
            VS = sb.tile([M, D], F32)
            ONES = sb.tile([M, M], F32)
            nc.sync.dma_start(CT[:], ct_d)
            nc.sync.dma_start(AT[:], at_d)
            nc.sync.dma_start(BT[:], bt_d)
            nc.sync.dma_start(VS[:], vs_d)
            nc.gpsimd.memset(ONES[:], 1.0)

            T = sb.tile([M, N], F32)
            Z = sb.tile([M, N], F32)

            # t_1 = -C^T ;  z_1 = clip(t_1)
            nc.vector.tensor_scalar(T[:], CT[:], -1.0, None,
                                    mybir.AluOpType.mult)
            nc.vector.tensor_scalar(Z[:], T[:], 0.0, 1.0,
                                    mybir.AluOpType.max, mybir.AluOpType.min)

            for _ in range(N_ITERS - 1):
                pss = [psp.tile([M, CHUNK], F32, tag=f"ps{c}", name=f"ps{c}")
                       for c in range(NCHUNKS)]
                for c in range(NCHUNKS):
                    sl = bass.ts(c, CHUNK)
                    nc.tensor.matmul(pss[c][:], _mm_ap(AT[:]), _mm_ap(Z[:, sl]),
                                     start=True, stop=False)
                for c in range(NCHUNKS):
                    sl = bass.ts(c, CHUNK)
                    nc.tensor.matmul(pss[c][:], _mm_ap(BT[:]), _mm_ap(T[:, sl]),
                                     start=False, stop=True)
                for c in range(NCHUNKS):
                    sl = bass.ts(c, CHUNK)
                    nc.vector.tensor_tensor(T[:, sl], pss[c][:], CT[:, sl],
                                            mybir.AluOpType.subtract)
                    nc.vector.tensor_scalar(Z[:, sl], T[:, sl], 0.0, 1.0,
                                            mybir.AluOpType.max,
                                            mybir.AluOpType.min)

            # xb^T = (t_50 > 0.5)  as 1.0 / 0.0
            XB = sb.tile([M, N], F32)
            nc.vector.tensor_scalar(XB[:], T[:], 0.5, None,
                                    mybir.AluOpType.is_gt)

            # numerator: Vs^T xb^T  -> [d, n]; denominator: colsums broadcast
            pvs = [psp.tile([M, CHUNK], F32, tag=f"ps{c}", name=f"pv{c}")
                   for c in range(NCHUNKS)]
            pcs = [psp.tile([M, CHUNK], F32, tag=f"ps{c}", name=f"pc{c}")
                   for c in range(NCHUNKS)]
            for c in range(NCHUNKS):
                sl = bass.ts(c, CHUNK)
                nc.tensor.matmul(pvs[c][:], _mm_ap(VS[:]), _mm_ap(XB[:, sl]),
                                 start=True, stop=True)
            for c in range(NCHUNKS):
                sl = bass.ts(c, CHUNK)
                nc.tensor.matmul(pcs[c][:], _mm_ap(ONES[:]), _mm_ap(XB[:, sl]),
                                 start=True, stop=True)

            DEN = sb.tile([M, N], F32)
            for c in range(NCHUNKS):
                sl = bass.ts(c, CHUNK)
                nc.vector.tensor_scalar(DEN[:, sl], pcs[c][:], 1e-10, None,
                                        mybir.AluOpType.add)
            # 1/den = exp(-ln(den)) on the scalar engine
            LNV = sb.tile([M, N], F32)
            nc.scalar.activation(LNV[:], DEN[:], mybir.ActivationFunctionType.Ln)
            REC = sb.tile([M, N], F32)
            nc.scalar.activation(REC[:], LNV[:],
                                 mybir.ActivationFunctionType.Exp, scale=-1.0)

            OUT = sb.tile([D, N], F32)
            for c in range(NCHUNKS):
                sl = bass.ts(c, CHUNK)
                nc.vector.tensor_tensor(OUT[:, sl], pvs[c][:], REC[:, sl],
                                        mybir.AluOpType.mult)
            nc.sync.dma_start(out_d, OUT[:])

    nc.compile()
    _compiled[key] = nc
    return nc


def _host_precompute(Q, V):
    """Per-batch constants in float64, cast to float32."""
    b = Q.shape[0]
    m = V.shape[1]
    in_maps = []
    for bi in range(b):
        Vs64 = V[bi].astype(np.float64) / m
        eye = np.eye(m)
        Q1 = 2.0 * (Vs64 @ Vs64.T)
        Minv = np.linalg.inv(Q1 + RHO * eye)
        A = 2.0 * Minv - eye
        Bm = eye - Minv
        W = -2.0 * (Minv @ Vs64)
        c0 = (LAMBDA / m) * Minv.sum(axis=1)
        CT = W @ Q[bi].astype(np.float64).T + c0[:, None]
        # matmul computes lhsT.T @ rhs -> pass explicit transposes
        in_maps.append({
            "ct": np.ascontiguousarray(CT, dtype=np.float32),
            "at": np.ascontiguousarray(A.T, dtype=np.float32),
            "bt": np.ascontiguousarray(Bm.T, dtype=np.float32),
            # final product lhsT = Vs (out^T = Vs^T @ xb^T); match the
            # reference's f32 V/m rounding exactly
            "vs": np.ascontiguousarray(V[bi].astype(np.float32) / np.float32(m)),
        })
    return in_maps


def kernel(Q, V):
    Q = np.asarray(Q, dtype=np.float32)
    V = np.asarray(V, dtype=np.float32)
    nc = _build()
    in_maps = _host_precompute(Q, V)
    res = run_bass_kernel_spmd(nc, in_maps, list(range(N_CORES)))
    out = np.empty((B, N, D), dtype=np.float32)
    for bi in range(B):
        out[bi] = res.results[bi]["outT"].T
    return out


# revision 9
# speedup vs baseline: 5.0053x; 1.0101x over previous
"""Trainium2 Bass kernel for nn_Attention_58437325029959 (sparse_attention).

Reference computation (per batch b, with m = d = 128, n = 2048):
    Vs = V / m
    Q1 = 2 Vs Vs^T;  P = -2 Vs Q^T + lam/m        (P viewed as [n, m])
    50 ADMM iterations of the box QP  min 0.5 x^T Q1 x + P x, 0 <= x <= 1
    xb = (z_50 > 0.5);  out = (xb / rowsum(xb)) @ Vs

Algebraic form used on device (exactly equivalent in exact arithmetic):
    M_inv = inv(Q1 + I);  A = 2 M_inv - I;  B = I - M_inv
    C^T   = (-2 M_inv Vs) Q^T + (lam/m) (M_inv 1) 1^T        [m, n]
    t_1   = -C^T;   t_{k+1} = A z_k + B t_k - C^T,  z_k = clip(t_k)
    out^T = (Vs^T xb^T) / colsum(xb^T),  xb^T = (t_50 > 0.5)

Sharding: one batch element per NeuronCore (8 cores).  All state is kept
transposed: [m=128 partitions, n=2048 free] per core.
"""

import numpy as np

import concourse.bass as bass
import concourse.mybir as mybir
import concourse.tile as tile
from concourse import bacc
from concourse.bass_utils import run_bass_kernel_spmd

LAMBDA = 0.1
RHO = 1.0
N_ITERS = 50

B, N, D = 8, 2048, 128
M = 128
N_CORES = 8
CHUNK = 512
NCHUNKS = N // CHUNK

F32 = mybir.dt.float32
F32R = mybir.dt.float32r

# 'f32' (exact, 4 cyc/row) or 'f32r' (1 cyc/row at N>=256, reduced precision)
import os as _os
MM_DTYPE = _os.environ.get("KERNEL_MM_DTYPE", "f32")

_compiled = {}


def _mm_ap(ap):
    if MM_DTYPE == "f32r":
        return ap.bitcast(F32R)
    return ap


def _build():
    """Build (and cache) the Bass program. Same program on all 8 cores."""
    key = (MM_DTYPE,)
    if key in _compiled:
        return _compiled[key]

    nc = bacc.Bacc("TRN2", target_bir_lowering=False, debug=False,
                   num_devices=N_CORES)

    ct_d = nc.dram_tensor("ct", [M, N], F32, kind="ExternalInput").ap()
    at_d = nc.dram_tensor("at", [M, M], F32, kind="ExternalInput").ap()
    bt_d = nc.dram_tensor("bt", [M, M], F32, kind="ExternalInput").ap()
    vs_d = nc.dram_tensor("vs", [M, D], F32, kind="ExternalInput").ap()
    out_d = nc.dram_tensor("outT", [D, N], F32, kind="ExternalOutput").ap()

    with tile.TileContext(nc) as tc:
        with (
            tc.tile_pool(name="sb", bufs=1) as sb,
            tc.tile_pool(name="ps", bufs=2, space="PSUM") as psp,
        ):
            CT = sb.tile([M, N], F32)
            AT = sb.tile([M, M], F32)
            BT = sb.tile([M, M], F32)
            VS = sb.tile([M, D], F32)
            ONES = sb.tile([M, M], F32)
            nc.sync.dma_start(AT[:], at_d)
            nc.sync.dma_start(BT[:], bt_d)
            for c in range(NCHUNKS):
                sl = bass.ts(c, CHUNK)
                nc.sync.dma_start(CT[:, sl], ct_d[:, sl])
            nc.sync.dma_start(VS[:], vs_d)
            nc.gpsimd.memset(ONES[:], 1.0)

            T = sb.tile([M, N], F32)
            Z = sb.tile([M, N], F32)

            # t_1 = -C^T ;  z_1 = clip(t_1)  (chunked so iter 1 starts early)
            for c in range(NCHUNKS):
                sl = bass.ts(c, CHUNK)
                nc.vector.tensor_scalar(T[:, sl], CT[:, sl], -1.0, None,
                                        mybir.AluOpType.mult)
                nc.vector.tensor_scalar(Z[:, sl], T[:, sl], 0.0, 1.0,
                                        mybir.AluOpType.max,
                                        mybir.AluOpType.min)

            for it in range(N_ITERS - 1):
                last = it == N_ITERS - 2
                pss = [psp.tile([M, CHUNK], F32, tag=f"ps{c}", name=f"ps{c}")
                       for c in range(NCHUNKS)]
                for c in range(NCHUNKS):
                    sl = bass.ts(c, CHUNK)
                    nc.tensor.matmul(pss[c][:], _mm_ap(AT[:]), _mm_ap(Z[:, sl]),
                                     start=True, stop=False)
                for c in range(NCHUNKS):
                    sl = bass.ts(c, CHUNK)
                    nc.tensor.matmul(pss[c][:], _mm_ap(BT[:]), _mm_ap(T[:, sl]),
                                     start=False, stop=True)
                for c in range(NCHUNKS):
                    sl = bass.ts(c, CHUNK)
                    nc.vector.tensor_tensor(T[:, sl], pss[c][:], CT[:, sl],
                                            mybir.AluOpType.subtract)
                    if not last:
                        nc.vector.tensor_scalar(Z[:, sl], T[:, sl], 0.0, 1.0,
                                                mybir.AluOpType.max,
                                                mybir.AluOpType.min)

            # xb^T = (t_50 > 0.5)  as 1.0 / 0.0  (chunked; z_50 never needed)
            XB = sb.tile([M, N], F32)
            for c in range(NCHUNKS):
                sl = bass.ts(c, CHUNK)
                nc.vector.tensor_scalar(XB[:, sl], T[:, sl], 0.5, None,
                                        mybir.AluOpType.is_gt)

            # numerator: Vs^T xb^T  -> [d, n]; denominator: colsums broadcast
            pvs = [psp.tile([M, CHUNK], F32, tag=f"ps{c}", name=f"pv{c}")
                   for c in range(NCHUNKS)]
            pcs = [psp.tile([M, CHUNK], F32, tag=f"ps{c}", name=f"pc{c}")
                   for c in range(NCHUNKS)]
            for c in range(NCHUNKS):
                sl = bass.ts(c, CHUNK)
                nc.tensor.matmul(pvs[c][:], _mm_ap(VS[:]), _mm_ap(XB[:, sl]),
                                 start=True, stop=True)
            for c in range(NCHUNKS):
                sl = bass.ts(c, CHUNK)
                nc.tensor.matmul(pcs[c][:], _mm_ap(ONES[:]), _mm_ap(XB[:, sl]),
                                 start=True, stop=True)

            DEN = sb.tile([M, N], F32)
            for c in range(NCHUNKS):
                sl = bass.ts(c, CHUNK)
                nc.vector.tensor_scalar(DEN[:, sl], pcs[c][:], 1e-10, None,
                                        mybir.AluOpType.add)
            # 1/den = exp(-ln(den)) on the scalar engine
            LNV = sb.tile([M, N], F32)
            nc.scalar.activation(LNV[:], DEN[:], mybir.ActivationFunctionType.Ln)
            REC = sb.tile([M, N], F32)
            nc.scalar.activation(REC[:], LNV[:],
                                 mybir.ActivationFunctionType.Exp, scale=-1.0)

            OUT = sb.tile([D, N], F32)
            for c in range(NCHUNKS):
                sl = bass.ts(c, CHUNK)
                nc.vector.tensor_tensor(OUT[:, sl], pvs[c][:], REC[:, sl],
                                        mybir.AluOpType.mult)
            nc.sync.dma_start(out_d, OUT[:])

    nc.compile()
    _compiled[key] = nc
    return nc


def _host_precompute(Q, V):
    """Per-batch constants in float64, cast to float32."""
    b = Q.shape[0]
    m = V.shape[1]
    in_maps = []
    for bi in range(b):
        Vs64 = V[bi].astype(np.float64) / m
        eye = np.eye(m)
        Q1 = 2.0 * (Vs64 @ Vs64.T)
        Minv = np.linalg.inv(Q1 + RHO * eye)
        A = 2.0 * Minv - eye
        Bm = eye - Minv
        W = -2.0 * (Minv @ Vs64)
        c0 = (LAMBDA / m) * Minv.sum(axis=1)
        CT = W @ Q[bi].astype(np.float64).T + c0[:, None]
        # matmul computes lhsT.T @ rhs -> pass explicit transposes
        in_maps.append({
            "ct": np.ascontiguousarray(CT, dtype=np.float32),
            "at": np.ascontiguousarray(A.T, dtype=np.float32),
            "bt": np.ascontiguousarray(Bm.T, dtype=np.float32),
            # final product lhsT = Vs (out^T = Vs^T @ xb^T); match the
            # reference's f32 V/m rounding exactly
            "vs": np.ascontiguousarray(V[bi].astype(np.float32) / np.float32(m)),
        })
    return in_maps


def kernel(Q, V):
    Q = np.asarray(Q, dtype=np.float32)
    V = np.asarray(V, dtype=np.float32)
    nc = _build()
    in_maps = _host_precompute(Q, V)
    res = run_bass_kernel_spmd(nc, in_maps, list(range(N_CORES)))
    out = np.empty((B, N, D), dtype=np.float32)
    for bi in range(B):
        out[bi] = res.results[bi]["outT"].T
    return out


# revision 11
# speedup vs baseline: 5.0817x; 1.0153x over previous
"""Trainium2 Bass kernel for nn_Attention_58437325029959 (sparse_attention).

Reference computation (per batch b, with m = d = 128, n = 2048):
    Vs = V / m
    Q1 = 2 Vs Vs^T;  P = -2 Vs Q^T + lam/m        (P viewed as [n, m])
    50 ADMM iterations of the box QP  min 0.5 x^T Q1 x + P x, 0 <= x <= 1
    xb = (z_50 > 0.5);  out = (xb / rowsum(xb)) @ Vs

Algebraic form used on device (exactly equivalent in exact arithmetic):
    M_inv = inv(Q1 + I);  A = 2 M_inv - I;  B = I - M_inv
    C^T   = (-2 M_inv Vs) Q^T + (lam/m) (M_inv 1) 1^T        [m, n]
    t_1   = -C^T;   t_{k+1} = A z_k + B t_k - C^T,  z_k = clip(t_k)
    out^T = (Vs^T xb^T) / colsum(xb^T),  xb^T = (t_50 > 0.5)

Sharding: one batch element per NeuronCore (8 cores).  All state is kept
transposed: [m=128 partitions, n=2048 free] per core.
"""

import numpy as np

import concourse.bass as bass
import concourse.mybir as mybir
import concourse.tile as tile
from concourse import bacc
from concourse.bass_utils import run_bass_kernel_spmd

LAMBDA = 0.1
RHO = 1.0
N_ITERS = 50

B, N, D = 8, 2048, 128
M = 128
N_CORES = 8
CHUNK = 512
NCHUNKS = N // CHUNK

F32 = mybir.dt.float32
F32R = mybir.dt.float32r

# 'f32' (exact, 4 cyc/row) or 'f32r' (1 cyc/row at N>=256, reduced precision)
import os as _os
MM_DTYPE = _os.environ.get("KERNEL_MM_DTYPE", "f32")

_compiled = {}


def _mm_ap(ap):
    if MM_DTYPE == "f32r":
        return ap.bitcast(F32R)
    return ap


def _build():
    """Build (and cache) the Bass program. Same program on all 8 cores."""
    key = (MM_DTYPE,)
    if key in _compiled:
        return _compiled[key]

    nc = bacc.Bacc("TRN2", target_bir_lowering=False, debug=False,
                   num_devices=N_CORES)

    ct_d = nc.dram_tensor("ct", [M, N], F32, kind="ExternalInput").ap()
    at_d = nc.dram_tensor("at", [M, M], F32, kind="ExternalInput").ap()
    bt_d = nc.dram_tensor("bt", [M, M], F32, kind="ExternalInput").ap()
    vs_d = nc.dram_tensor("vs", [M, D], F32, kind="ExternalInput").ap()
    out_d = nc.dram_tensor("outT", [D, N], F32, kind="ExternalOutput").ap()

    with tile.TileContext(nc) as tc:
        with (
            tc.tile_pool(name="sb", bufs=1) as sb,
            tc.tile_pool(name="ps", bufs=2, space="PSUM") as psp,
        ):
            CT = sb.tile([M, N], F32)
            AT = sb.tile([M, M], F32)
            BT = sb.tile([M, M], F32)
            VS = sb.tile([M, D], F32)
            ONES = sb.tile([M, M], F32)
            nc.sync.dma_start(CT[:, bass.ts(0, CHUNK)], ct_d[:, bass.ts(0, CHUNK)])
            nc.sync.dma_start(AT[:], at_d)
            nc.sync.dma_start(BT[:], bt_d)
            for c in range(1, NCHUNKS):
                sl = bass.ts(c, CHUNK)
                nc.sync.dma_start(CT[:, sl], ct_d[:, sl])
            nc.sync.dma_start(VS[:], vs_d)
            nc.gpsimd.memset(ONES[:], 1.0)

            T = sb.tile([M, N], F32)
            Z = sb.tile([M, N], F32)

            # Preload the Ln/Exp activation tables so the epilogue doesn't
            # stall on ACT_TABLE_LOAD.
            WARM = sb.tile([M, 1], F32)
            nc.gpsimd.memset(WARM[:], 1.0)
            nc.scalar.activation(WARM[:], WARM[:], mybir.ActivationFunctionType.Ln)
            nc.scalar.activation(WARM[:], WARM[:], mybir.ActivationFunctionType.Exp)

            # t_1 = -C^T ;  z_1 = clip(t_1)  (chunked so iter 1 starts early)
            for c in range(NCHUNKS):
                sl = bass.ts(c, CHUNK)
                nc.vector.tensor_scalar(T[:, sl], CT[:, sl], -1.0, None,
                                        mybir.AluOpType.mult)
                nc.vector.tensor_scalar(Z[:, sl], T[:, sl], 0.0, 1.0,
                                        mybir.AluOpType.max,
                                        mybir.AluOpType.min)

            for it in range(N_ITERS - 1):
                last = it == N_ITERS - 2
                pss = [psp.tile([M, CHUNK], F32, tag=f"ps{c}", name=f"ps{c}")
                       for c in range(NCHUNKS)]
                for c in range(NCHUNKS):
                    sl = bass.ts(c, CHUNK)
                    nc.tensor.matmul(pss[c][:], _mm_ap(AT[:]), _mm_ap(Z[:, sl]),
                                     start=True, stop=False)
                for c in range(NCHUNKS):
                    sl = bass.ts(c, CHUNK)
                    nc.tensor.matmul(pss[c][:], _mm_ap(BT[:]), _mm_ap(T[:, sl]),
                                     start=False, stop=True)
                for c in range(NCHUNKS):
                    sl = bass.ts(c, CHUNK)
                    nc.vector.tensor_tensor(T[:, sl], pss[c][:], CT[:, sl],
                                            mybir.AluOpType.subtract)
                    if not last:
                        nc.vector.tensor_scalar(Z[:, sl], T[:, sl], 0.0, 1.0,
                                                mybir.AluOpType.max,
                                                mybir.AluOpType.min)

            # xb^T = (t_50 > 0.5)  as 1.0 / 0.0  (chunked; z_50 never needed)
            XB = sb.tile([M, N], F32)
            for c in range(NCHUNKS):
                sl = bass.ts(c, CHUNK)
                nc.vector.tensor_scalar(XB[:, sl], T[:, sl], 0.5, None,
                                        mybir.AluOpType.is_gt)

            # numerator: Vs^T xb^T  -> [d, n]; denominator: colsums broadcast
            pvs = [psp.tile([M, CHUNK], F32, tag=f"ps{c}", name=f"pv{c}")
                   for c in range(NCHUNKS)]
            pcs = [psp.tile([M, CHUNK], F32, tag=f"ps{c}", name=f"pc{c}")
                   for c in range(NCHUNKS)]
            for c in range(NCHUNKS):
                sl = bass.ts(c, CHUNK)
                nc.tensor.matmul(pvs[c][:], _mm_ap(VS[:]), _mm_ap(XB[:, sl]),
                                 start=True, stop=True)
            for c in range(NCHUNKS):
                sl = bass.ts(c, CHUNK)
                nc.tensor.matmul(pcs[c][:], _mm_ap(ONES[:]), _mm_ap(XB[:, sl]),
                                 start=True, stop=True)

            DEN = sb.tile([M, N], F32)
            for c in range(NCHUNKS):
                sl = bass.ts(c, CHUNK)
                nc.vector.tensor_scalar(DEN[:, sl], pcs[c][:], 1e-10, None,
                                        mybir.AluOpType.add)
            # 1/den = exp(-ln(den)) on the scalar engine
            LNV = sb.tile([M, N], F32)
            nc.scalar.activation(LNV[:], DEN[:], mybir.ActivationFunctionType.Ln)
            REC = sb.tile([M, N], F32)
            nc.scalar.activation(REC[:], LNV[:],
                                 mybir.ActivationFunctionType.Exp, scale=-1.0)

            OUT = sb.tile([D, N], F32)
            for c in range(NCHUNKS):
                sl = bass.ts(c, CHUNK)
                nc.vector.tensor_tensor(OUT[:, sl], pvs[c][:], REC[:, sl],
                                        mybir.AluOpType.mult)
                nc.sync.dma_start(out_d[:, sl], OUT[:, sl])

    nc.compile()
    _compiled[key] = nc
    return nc


def _host_precompute(Q, V):
    """Per-batch constants in float64, cast to float32."""
    b = Q.shape[0]
    m = V.shape[1]
    in_maps = []
    for bi in range(b):
        Vs64 = V[bi].astype(np.float64) / m
        eye = np.eye(m)
        Q1 = 2.0 * (Vs64 @ Vs64.T)
        Minv = np.linalg.inv(Q1 + RHO * eye)
        A = 2.0 * Minv - eye
        Bm = eye - Minv
        W = -2.0 * (Minv @ Vs64)
        c0 = (LAMBDA / m) * Minv.sum(axis=1)
        CT = W @ Q[bi].astype(np.float64).T + c0[:, None]
        # matmul computes lhsT.T @ rhs -> pass explicit transposes
        in_maps.append({
            "ct": np.ascontiguousarray(CT, dtype=np.float32),
            "at": np.ascontiguousarray(A.T, dtype=np.float32),
            "bt": np.ascontiguousarray(Bm.T, dtype=np.float32),
            # final product lhsT = Vs (out^T = Vs^T @ xb^T); match the
            # reference's f32 V/m rounding exactly
            "vs": np.ascontiguousarray(V[bi].astype(np.float32) / np.float32(m)),
        })
    return in_maps


def kernel(Q, V):
    Q = np.asarray(Q, dtype=np.float32)
    V = np.asarray(V, dtype=np.float32)
    nc = _build()
    in_maps = _host_precompute(Q, V)
    res = run_bass_kernel_spmd(nc, in_maps, list(range(N_CORES)))
    out = np.empty((B, N, D), dtype=np.float32)
    for bi in range(B):
        out[bi] = res.results[bi]["outT"].T
    return out


# revision 14
# speedup vs baseline: 5.0839x; 1.0005x over previous
"""Trainium2 Bass kernel for nn_Attention_58437325029959 (sparse_attention).

Reference computation (per batch b, with m = d = 128, n = 2048):
    Vs = V / m
    Q1 = 2 Vs Vs^T;  P = -2 Vs Q^T + lam/m        (P viewed as [n, m])
    50 ADMM iterations of the box QP  min 0.5 x^T Q1 x + P x, 0 <= x <= 1
    xb = (z_50 > 0.5);  out = (xb / rowsum(xb)) @ Vs

Algebraic form used on device (exactly equivalent in exact arithmetic):
    M_inv = inv(Q1 + I);  A = 2 M_inv - I;  B = I - M_inv
    C^T   = (-2 M_inv Vs) Q^T + (lam/m) (M_inv 1) 1^T        [m, n]
    t_1   = -C^T;   t_{k+1} = A z_k + B t_k - C^T,  z_k = clip(t_k)
    out^T = (Vs^T xb^T) / colsum(xb^T),  xb^T = (t_50 > 0.5)

Sharding: one batch element per NeuronCore (8 cores).  All state is kept
transposed: [m=128 partitions, n=2048 free] per core.
"""

import numpy as np

import concourse.bass as bass
import concourse.mybir as mybir
import concourse.tile as tile
from concourse import bacc
from concourse.bass_utils import run_bass_kernel_spmd

LAMBDA = 0.1
RHO = 1.0
N_ITERS = 50

B, N, D = 8, 2048, 128
M = 128
N_CORES = 8
CHUNK = 512
NCHUNKS = N // CHUNK

F32 = mybir.dt.float32
F32R = mybir.dt.float32r

# 'f32' (exact, 4 cyc/row) or 'f32r' (1 cyc/row at N>=256, reduced precision)
import os as _os
MM_DTYPE = _os.environ.get("KERNEL_MM_DTYPE", "f32")

_compiled = {}


def _mm_ap(ap):
    if MM_DTYPE == "f32r":
        return ap.bitcast(F32R)
    return ap


def _build():
    """Build (and cache) the Bass program. Same program on all 8 cores."""
    key = (MM_DTYPE,)
    if key in _compiled:
        return _compiled[key]

    nc = bacc.Bacc("TRN2", target_bir_lowering=False, debug=False,
                   num_devices=N_CORES)

    ct_d = nc.dram_tensor("ct", [M, N], F32, kind="ExternalInput").ap()
    ctx_d = nc.dram_tensor("ctx", [M, N], F32, kind="ExternalInput").ap()
    at_d = nc.dram_tensor("at", [M, M], F32, kind="ExternalInput").ap()
    bt_d = nc.dram_tensor("bt", [M, M], F32, kind="ExternalInput").ap()
    vs_d = nc.dram_tensor("vs", [M, D], F32, kind="ExternalInput").ap()
    out_d = nc.dram_tensor("outT", [D, N], F32, kind="ExternalOutput").ap()

    with tile.TileContext(nc) as tc:
        with (
            tc.tile_pool(name="sb", bufs=1) as sb,
            tc.tile_pool(name="ps", bufs=2, space="PSUM") as psp,
        ):
            CT = sb.tile([M, N], F32)
            CTX = sb.tile([M, N], F32)
            AT = sb.tile([M, M], F32)
            BT = sb.tile([M, M], F32)
            VS = sb.tile([M, D], F32)
            ONES = sb.tile([M, M], F32)
            nc.sync.dma_start(CT[:, bass.ts(0, CHUNK)], ct_d[:, bass.ts(0, CHUNK)])
            nc.sync.dma_start(AT[:], at_d)
            nc.sync.dma_start(BT[:], bt_d)
            for c in range(1, NCHUNKS):
                sl = bass.ts(c, CHUNK)
                nc.sync.dma_start(CT[:, sl], ct_d[:, sl])
            nc.sync.dma_start(CTX[:], ctx_d)
            nc.sync.dma_start(VS[:], vs_d)
            nc.gpsimd.memset(ONES[:], 1.0)

            T = sb.tile([M, N], F32)
            Z = sb.tile([M, N], F32)

            # Preload the Ln/Exp activation tables so the epilogue doesn't
            # stall on ACT_TABLE_LOAD.
            WARM = sb.tile([M, 1], F32)
            nc.gpsimd.memset(WARM[:], 1.0)
            nc.scalar.activation(WARM[:], WARM[:], mybir.ActivationFunctionType.Ln)
            nc.scalar.activation(WARM[:], WARM[:], mybir.ActivationFunctionType.Exp)

            # z_1 = clip(-C^T); t_1 = -C^T is folded into ctx = C^T + B C^T
            # so iteration 1 needs no B-product and no explicit t_1.
            for c in range(NCHUNKS):
                sl = bass.ts(c, CHUNK)
                nc.vector.tensor_scalar(Z[:, sl], CT[:, sl], -1.0, 0.0,
                                        mybir.AluOpType.mult,
                                        mybir.AluOpType.max)
                nc.vector.tensor_scalar(Z[:, sl], Z[:, sl], 1.0, None,
                                        mybir.AluOpType.min)

            for it in range(N_ITERS - 1):
                first = it == 0
                last = it == N_ITERS - 2
                pss = [psp.tile([M, CHUNK], F32, tag=f"ps{c}", name=f"ps{c}")
                       for c in range(NCHUNKS)]
                for c in range(NCHUNKS):
                    sl = bass.ts(c, CHUNK)
                    nc.tensor.matmul(pss[c][:], _mm_ap(AT[:]), _mm_ap(Z[:, sl]),
                                     start=True, stop=first)
                if not first:
                    for c in range(NCHUNKS):
                        sl = bass.ts(c, CHUNK)
                        nc.tensor.matmul(pss[c][:], _mm_ap(BT[:]),
                                         _mm_ap(T[:, sl]),
                                         start=False, stop=True)
                CREF = CTX if first else CT
                for c in range(NCHUNKS):
                    sl = bass.ts(c, CHUNK)
                    nc.vector.tensor_tensor(T[:, sl], pss[c][:], CREF[:, sl],
                                            mybir.AluOpType.subtract)
                    if not last:
                        nc.vector.tensor_scalar(Z[:, sl], T[:, sl], 0.0, 1.0,
                                                mybir.AluOpType.max,
                                                mybir.AluOpType.min)

            # xb^T = (t_50 > 0.5)  as 1.0 / 0.0  (chunked; z_50 never needed)
            XB = sb.tile([M, N], F32)
            for c in range(NCHUNKS):
                sl = bass.ts(c, CHUNK)
                nc.vector.tensor_scalar(XB[:, sl], T[:, sl], 0.5, None,
                                        mybir.AluOpType.is_gt)

            # numerator: Vs^T xb^T  -> [d, n]; denominator: colsums broadcast
            pvs = [psp.tile([M, CHUNK], F32, tag=f"ps{c}", name=f"pv{c}")
                   for c in range(NCHUNKS)]
            pcs = [psp.tile([M, CHUNK], F32, tag=f"ps{c}", name=f"pc{c}")
                   for c in range(NCHUNKS)]
            for c in range(NCHUNKS):
                sl = bass.ts(c, CHUNK)
                nc.tensor.matmul(pvs[c][:], _mm_ap(VS[:]), _mm_ap(XB[:, sl]),
                                 start=True, stop=True)
            for c in range(NCHUNKS):
                sl = bass.ts(c, CHUNK)
                nc.tensor.matmul(pcs[c][:], _mm_ap(ONES[:]), _mm_ap(XB[:, sl]),
                                 start=True, stop=True)

            DEN = sb.tile([M, N], F32)
            for c in range(NCHUNKS):
                sl = bass.ts(c, CHUNK)
                nc.vector.tensor_scalar(DEN[:, sl], pcs[c][:], 1e-10, None,
                                        mybir.AluOpType.add)
            # 1/den = exp(-ln(den)) on the scalar engine
            LNV = sb.tile([M, N], F32)
            nc.scalar.activation(LNV[:], DEN[:], mybir.ActivationFunctionType.Ln)
            REC = sb.tile([M, N], F32)
            nc.scalar.activation(REC[:], LNV[:],
                                 mybir.ActivationFunctionType.Exp, scale=-1.0)

            OUT = sb.tile([D, N], F32)
            for c in range(NCHUNKS):
                sl = bass.ts(c, CHUNK)
                nc.vector.tensor_tensor(OUT[:, sl], pvs[c][:], REC[:, sl],
                                        mybir.AluOpType.mult)
                nc.sync.dma_start(out_d[:, sl], OUT[:, sl])

    nc.compile()
    _compiled[key] = nc
    return nc


def _host_precompute(Q, V):
    """Per-batch constants in float64, cast to float32."""
    b = Q.shape[0]
    m = V.shape[1]
    in_maps = []
    for bi in range(b):
        Vs64 = V[bi].astype(np.float64) / m
        eye = np.eye(m)
        Q1 = 2.0 * (Vs64 @ Vs64.T)
        Minv = np.linalg.inv(Q1 + RHO * eye)
        A = 2.0 * Minv - eye
        Bm = eye - Minv
        W = -2.0 * (Minv @ Vs64)
        c0 = (LAMBDA / m) * Minv.sum(axis=1)
        CT = W @ Q[bi].astype(np.float64).T + c0[:, None]
        CTX = CT + Bm @ CT  # iteration-1 fold: t_2 = A z_1 - (C^T + B C^T)
        # matmul computes lhsT.T @ rhs -> pass explicit transposes
        in_maps.append({
            "ct": np.ascontiguousarray(CT, dtype=np.float32),
            "ctx": np.ascontiguousarray(CTX, dtype=np.float32),
            "at": np.ascontiguousarray(A.T, dtype=np.float32),
            "bt": np.ascontiguousarray(Bm.T, dtype=np.float32),
            # final product lhsT = Vs (out^T = Vs^T @ xb^T); match the
            # reference's f32 V/m rounding exactly
            "vs": np.ascontiguousarray(V[bi].astype(np.float32) / np.float32(m)),
        })
    return in_maps


def kernel(Q, V):
    Q = np.asarray(Q, dtype=np.float32)
    V = np.asarray(V, dtype=np.float32)
    nc = _build()
    in_maps = _host_precompute(Q, V)
    res = run_bass_kernel_spmd(nc, in_maps, list(range(N_CORES)))
    out = np.empty((B, N, D), dtype=np.float32)
    for bi in range(B):
        out[bi] = res.results[bi]["outT"].T
    return out


# revision 15
# speedup vs baseline: 5.0888x; 1.0010x over previous
"""Trainium2 Bass kernel for nn_Attention_58437325029959 (sparse_attention).

Reference computation (per batch b, with m = d = 128, n = 2048):
    Vs = V / m
    Q1 = 2 Vs Vs^T;  P = -2 Vs Q^T + lam/m        (P viewed as [n, m])
    50 ADMM iterations of the box QP  min 0.5 x^T Q1 x + P x, 0 <= x <= 1
    xb = (z_50 > 0.5);  out = (xb / rowsum(xb)) @ Vs

Algebraic form used on device (exactly equivalent in exact arithmetic):
    M_inv = inv(Q1 + I);  A = 2 M_inv - I;  B = I - M_inv
    C^T   = (-2 M_inv Vs) Q^T + (lam/m) (M_inv 1) 1^T        [m, n]
    t_1   = -C^T;   t_{k+1} = A z_k + B t_k - C^T,  z_k = clip(t_k)
    out^T = (Vs^T xb^T) / colsum(xb^T),  xb^T = (t_50 > 0.5)

Sharding: one batch element per NeuronCore (8 cores).  All state is kept
transposed: [m=128 partitions, n=2048 free] per core.
"""

import numpy as np

import concourse.bass as bass
import concourse.mybir as mybir
import concourse.tile as tile
from concourse import bacc
from concourse.bass_utils import run_bass_kernel_spmd

LAMBDA = 0.1
RHO = 1.0
N_ITERS = 50

B, N, D = 8, 2048, 128
M = 128
N_CORES = 8
CHUNK = 512
NCHUNKS = N // CHUNK

F32 = mybir.dt.float32

# All matmuls run in exact fp32 (4 cyc/row on the PE): the ADMM selection
# margins go down to ~6e-6, and perturbation experiments show noise >=3e-6
# per iteration flips selections, so bf16/fp32r products are not usable.
MM_DTYPE = "f32"

_compiled = {}


def _mm_ap(ap):
    return ap


def _build():
    """Build (and cache) the Bass program. Same program on all 8 cores."""
    key = "k"
    if key in _compiled:
        return _compiled[key]

    nc = bacc.Bacc("TRN2", target_bir_lowering=False, debug=False,
                   num_devices=N_CORES)

    ct_d = nc.dram_tensor("ct", [M, N], F32, kind="ExternalInput").ap()
    ctx_d = nc.dram_tensor("ctx", [M, N], F32, kind="ExternalInput").ap()
    at_d = nc.dram_tensor("at", [M, M], F32, kind="ExternalInput").ap()
    bt_d = nc.dram_tensor("bt", [M, M], F32, kind="ExternalInput").ap()
    vs_d = nc.dram_tensor("vs", [M, D], F32, kind="ExternalInput").ap()
    out_d = nc.dram_tensor("outT", [D, N], F32, kind="ExternalOutput").ap()

    with tile.TileContext(nc) as tc:
        with (
            tc.tile_pool(name="sb", bufs=1) as sb,
            tc.tile_pool(name="ps", bufs=2, space="PSUM") as psp,
        ):
            CT = sb.tile([M, N], F32)
            CTX = sb.tile([M, N], F32)
            AT = sb.tile([M, M], F32)
            BT = sb.tile([M, M], F32)
            VS = sb.tile([M, D], F32)
            ONES = sb.tile([M, M], F32)
            nc.sync.dma_start(CT[:, bass.ts(0, CHUNK)], ct_d[:, bass.ts(0, CHUNK)])
            nc.sync.dma_start(AT[:], at_d)
            nc.sync.dma_start(BT[:], bt_d)
            for c in range(1, NCHUNKS):
                sl = bass.ts(c, CHUNK)
                nc.sync.dma_start(CT[:, sl], ct_d[:, sl])
            nc.sync.dma_start(CTX[:], ctx_d)
            nc.sync.dma_start(VS[:], vs_d)
            nc.gpsimd.memset(ONES[:], 1.0)

            T = sb.tile([M, N], F32)
            Z = sb.tile([M, N], F32)

            # Preload the Ln/Exp activation tables so the epilogue doesn't
            # stall on ACT_TABLE_LOAD.
            WARM = sb.tile([M, 1], F32)
            nc.gpsimd.memset(WARM[:], 1.0)
            nc.scalar.activation(WARM[:], WARM[:], mybir.ActivationFunctionType.Ln)
            nc.scalar.activation(WARM[:], WARM[:], mybir.ActivationFunctionType.Exp)

            # z_1 = clip(-C^T); t_1 = -C^T is folded into ctx = C^T + B C^T
            # so iteration 1 needs no B-product and no explicit t_1.
            for c in range(NCHUNKS):
                sl = bass.ts(c, CHUNK)
                nc.vector.tensor_scalar(Z[:, sl], CT[:, sl], -1.0, 0.0,
                                        mybir.AluOpType.mult,
                                        mybir.AluOpType.max)
                nc.vector.tensor_scalar(Z[:, sl], Z[:, sl], 1.0, None,
                                        mybir.AluOpType.min)

            for it in range(N_ITERS - 1):
                first = it == 0
                last = it == N_ITERS - 2
                pss = [psp.tile([M, CHUNK], F32, tag=f"ps{c}", name=f"ps{c}")
                       for c in range(NCHUNKS)]
                for c in range(NCHUNKS):
                    sl = bass.ts(c, CHUNK)
                    nc.tensor.matmul(pss[c][:], _mm_ap(AT[:]), _mm_ap(Z[:, sl]),
                                     start=True, stop=first)
                if not first:
                    for c in range(NCHUNKS):
                        sl = bass.ts(c, CHUNK)
                        nc.tensor.matmul(pss[c][:], _mm_ap(BT[:]),
                                         _mm_ap(T[:, sl]),
                                         start=False, stop=True)
                CREF = CTX if first else CT
                for c in range(NCHUNKS):
                    sl = bass.ts(c, CHUNK)
                    nc.vector.tensor_tensor(T[:, sl], pss[c][:], CREF[:, sl],
                                            mybir.AluOpType.subtract)
                    if not last:
                        nc.vector.tensor_scalar(Z[:, sl], T[:, sl], 0.0, 1.0,
                                                mybir.AluOpType.max,
                                                mybir.AluOpType.min)

            # xb^T = (t_50 > 0.5)  as 1.0 / 0.0  (chunked; z_50 never needed)
            XB = sb.tile([M, N], F32)
            for c in range(NCHUNKS):
                sl = bass.ts(c, CHUNK)
                nc.vector.tensor_scalar(XB[:, sl], T[:, sl], 0.5, None,
                                        mybir.AluOpType.is_gt)

            # numerator: Vs^T xb^T  -> [d, n]; denominator: colsums broadcast
            pvs = [psp.tile([M, CHUNK], F32, tag=f"ps{c}", name=f"pv{c}")
                   for c in range(NCHUNKS)]
            pcs = [psp.tile([M, CHUNK], F32, tag=f"ps{c}", name=f"pc{c}")
                   for c in range(NCHUNKS)]
            for c in range(NCHUNKS):
                sl = bass.ts(c, CHUNK)
                nc.tensor.matmul(pvs[c][:], _mm_ap(VS[:]), _mm_ap(XB[:, sl]),
                                 start=True, stop=True)
            for c in range(NCHUNKS):
                sl = bass.ts(c, CHUNK)
                nc.tensor.matmul(pcs[c][:], _mm_ap(ONES[:]), _mm_ap(XB[:, sl]),
                                 start=True, stop=True)

            DEN = sb.tile([M, N], F32)
            for c in range(NCHUNKS):
                sl = bass.ts(c, CHUNK)
                nc.vector.tensor_scalar(DEN[:, sl], pcs[c][:], 1e-10, None,
                                        mybir.AluOpType.add)
            # 1/den = exp(-ln(den)) on the scalar engine
            LNV = sb.tile([M, N], F32)
            nc.scalar.activation(LNV[:], DEN[:], mybir.ActivationFunctionType.Ln)
            REC = sb.tile([M, N], F32)
            nc.scalar.activation(REC[:], LNV[:],
                                 mybir.ActivationFunctionType.Exp, scale=-1.0)

            OUT = sb.tile([D, N], F32)
            for c in range(NCHUNKS):
                sl = bass.ts(c, CHUNK)
                nc.vector.tensor_tensor(OUT[:, sl], pvs[c][:], REC[:, sl],
                                        mybir.AluOpType.mult)
                nc.sync.dma_start(out_d[:, sl], OUT[:, sl])

    nc.compile()
    _compiled[key] = nc
    return nc


def _host_precompute(Q, V):
    """Per-batch constants in float64, cast to float32."""
    b = Q.shape[0]
    m = V.shape[1]
    in_maps = []
    for bi in range(b):
        Vs64 = V[bi].astype(np.float64) / m
        eye = np.eye(m)
        Q1 = 2.0 * (Vs64 @ Vs64.T)
        Minv = np.linalg.inv(Q1 + RHO * eye)
        A = 2.0 * Minv - eye
        Bm = eye - Minv
        W = -2.0 * (Minv @ Vs64)
        c0 = (LAMBDA / m) * Minv.sum(axis=1)
        CT = W @ Q[bi].astype(np.float64).T + c0[:, None]
        CTX = CT + Bm @ CT  # iteration-1 fold: t_2 = A z_1 - (C^T + B C^T)
        # matmul computes lhsT.T @ rhs -> pass explicit transposes
        in_maps.append({
            "ct": np.ascontiguousarray(CT, dtype=np.float32),
            "ctx": np.ascontiguousarray(CTX, dtype=np.float32),
            "at": np.ascontiguousarray(A.T, dtype=np.float32),
            "bt": np.ascontiguousarray(Bm.T, dtype=np.float32),
            # final product lhsT = Vs (out^T = Vs^T @ xb^T); match the
            # reference's f32 V/m rounding exactly
            "vs": np.ascontiguousarray(V[bi].astype(np.float32) / np.float32(m)),
        })
    return in_maps


def kernel(Q, V):
    Q = np.asarray(Q, dtype=np.float32)
    V = np.asarray(V, dtype=np.float32)
    nc = _build()
    in_maps = _host_precompute(Q, V)
    res = run_bass_kernel_spmd(nc, in_maps, list(range(N_CORES)))
    out = np.empty((B, N, D), dtype=np.float32)
    for bi in range(B):
        out[bi] = res.results[bi]["outT"].T
    return out
